# revision 2
# baseline (speedup 1.0000x reference)
"""Trainium2 kernel for a fuzzy-logic ConjunctionLayer forward pass.

Computes  out = 1[ (1 - x) @ 1[W > 0.5] <= 0 ]  for
x: [8192, 4096] f32, W: [4096, 2048] f32 -> out: [8192, 2048] f32.

Sharding: data-parallel over the batch dim across 8 NeuronCores
(x shard [1024, 4096] per core, W replicated), outputs concatenated.

Device kernel (per core):
  s  = (x - 1) cast to bf16      (sign-exact: bf16 shares f32's exponent
                                  range, so a nonzero f32 never rounds to 0)
  Wb = 1[W > 0.5] cast to bf16   ({0,1} exact in bf16)
  acc = s^T.T @ Wb accumulated in f32 PSUM (sum of non-positive terms)
  out = 1[acc >= 0]              (acc == 0  <=>  (1-x)@Wb == 0  <=>  res <= 0)

The bf16 mantissa rounding cannot flip the classification: every product
term has the sign of its f32 counterpart and the f32 PSUM accumulation of
same-signed terms is exactly zero iff all terms are zero.

x is transposed on the host during sharding so the contraction dim lands
on the SBUF partition axis for both matmul operands.
"""

import numpy as np

import concourse.bass as bass
import concourse.mybir as mybir
import concourse.tile as tile
from concourse import bacc
from concourse.bass_utils import run_bass_kernel_spmd

BATCH, IN_DIM, N_RULES = 8192, 4096, 2048
N_CORES = 8
M_LOCAL = BATCH // N_CORES  # 1024 batch rows per core

P = 128          # SBUF partitions / matmul tile edge
N_TILE = 512     # moving free dim per matmul (= one f32 PSUM bank)
KT = IN_DIM // P         # 32 contraction tiles
MT = M_LOCAL // P        # 8 output-row tiles per core
NB = N_RULES // N_TILE   # 4 output-column blocks

F32 = mybir.dt.float32
BF16 = mybir.dt.bfloat16
ALU = mybir.AluOpType


def _body(tc: tile.TileContext, out: bass.AP, xT: bass.AP, w: bass.AP):
    nc = tc.nc
    with (
        tc.tile_pool(name="sb", bufs=1) as sb,
        tc.tile_pool(name="ps", bufs=4, space="PSUM") as ps,
    ):
        # Resident (x - 1) in bf16, one [128, M_LOCAL] tile per k-tile.
        s_tiles = []
        for k in range(KT):
            xt = sb.tile([P, M_LOCAL], F32, tag="xt", bufs=3, name=f"xt{k}")
            nc.sync.dma_start(xt[:], xT[k * P:(k + 1) * P, :])
            s = sb.tile([P, M_LOCAL], BF16, tag=f"s{k}", bufs=1, name=f"s{k}")
            nc.vector.tensor_scalar(s[:], xt[:], 1.0, None, ALU.subtract)
            s_tiles.append(s)

        for nb in range(NB):
            n0 = nb * N_TILE
            # Binarized W block for these output columns, double-buffered
            # across nb so the next block loads during this block's matmuls.
            wb_tiles = []
            for k in range(KT):
                wf = sb.tile([P, N_TILE], F32, tag="wf", bufs=4, name=f"wf{nb}_{k}")
                nc.sync.dma_start(wf[:], w[k * P:(k + 1) * P, n0:n0 + N_TILE])
                wb = sb.tile([P, N_TILE], BF16, tag=f"wb{k}", bufs=2,
                             name=f"wb{nb}_{k}")
                nc.vector.tensor_scalar(wb[:], wf[:], 0.5, None, ALU.is_gt)
                wb_tiles.append(wb)

            for mt in range(MT):
                m0 = mt * P
                acc = ps.tile([P, N_TILE], F32, tag="acc", bufs=4,
                              name=f"acc{nb}_{mt}")
                for k in range(KT):
                    nc.tensor.matmul(
                        acc[:],
                        s_tiles[k][:, m0:m0 + P],
                        wb_tiles[k][:],
                        start=(k == 0),
                        stop=(k == KT - 1),
                    )
                o = sb.tile([P, N_TILE], F32, tag="o", bufs=4, name=f"o{nb}_{mt}")
                nc.vector.tensor_scalar(o[:], acc[:], 0.0, None, ALU.is_ge)
                nc.sync.dma_start(out[m0:m0 + P, n0:n0 + N_TILE], o[:])


_NC_CACHE = {}


def _get_nc():
    if "nc" not in _NC_CACHE:
        nc = bacc.Bacc("TRN2", target_bir_lowering=False, debug=False,
                       num_devices=N_CORES)
        xT = nc.dram_tensor("xT", [IN_DIM, M_LOCAL], F32, kind="ExternalInput")
        w = nc.dram_tensor("w", [IN_DIM, N_RULES], F32, kind="ExternalInput")
        out = nc.dram_tensor("out", [M_LOCAL, N_RULES], F32,
                             kind="ExternalOutput")
        with tile.TileContext(nc) as tc:
            _body(tc, out.ap(), xT.ap(), w.ap())
        nc.compile()
        _NC_CACHE["nc"] = nc
    return _NC_CACHE["nc"]


def kernel(x: np.ndarray, W: np.ndarray, **run_kwargs) -> np.ndarray:
    assert x.shape == (BATCH, IN_DIM) and W.shape == (IN_DIM, N_RULES)
    x = np.ascontiguousarray(x, dtype=np.float32)
    W = np.ascontiguousarray(W, dtype=np.float32)
    nc = _get_nc()
    in_maps = []
    for c in range(N_CORES):
        x_shard_t = np.ascontiguousarray(
            x[c * M_LOCAL:(c + 1) * M_LOCAL, :].T)  # [IN_DIM, M_LOCAL]
        in_maps.append({"xT": x_shard_t, "w": W})
    res = run_bass_kernel_spmd(nc, in_maps, core_ids=list(range(N_CORES)),
                               **run_kwargs)
    out = np.concatenate([res.results[c]["out"] for c in range(N_CORES)],
                         axis=0)
    if run_kwargs:
        kernel.last_results = res
    return out


# revision 3
# speedup vs baseline: 1.0576x; 1.0576x over previous
"""Trainium2 kernel for a fuzzy-logic ConjunctionLayer forward pass.

Computes  out = 1[ (1 - x) @ 1[W > 0.5] <= 0 ]  for
x: [8192, 4096] f32, W: [4096, 2048] f32 -> out: [8192, 2048] f32.

Sharding: data-parallel over the batch dim across 8 NeuronCores
(x shard [1024, 4096] per core, W replicated), outputs concatenated.

Math: with x in [0, 1], every term (1-x)*Wb is >= 0, so
  res[m,n] <= 0  <=>  res[m,n] == 0  <=>  no k has (x[m,k] < 1 AND W[k,n] > .5).
The output therefore depends only on the support pattern, so both operands
can be binarized on device:
  s  = 1[x < 1]   in {0,1}
  Wb = 1[W > .5]  in {0,1}
  acc = s^T.T @ Wb  (f32 PSUM accumulation of {0,1} products - exact)
  out = 1[acc <= 0]
{0,1} is exact in fp8e4, which unlocks the PE DoubleRow perf mode
(2 fp8 weights per cell => 2x MACs/cycle, contraction 256 per matmul).

Schedule (per core): N split into two 1024-wide blocks. The W block and
the first batch-chunk of x are streamed k-pair by k-pair so the first
accumulation chains start ~1us in and ride the DMA; remaining batch
chunks + the second W block prefetch under the compute. W loads issue on
the Sync HWDGE queue, x loads on the Scalar HWDGE queue, output stores
on GPSIMD SWDGE, so the three streams don't serialize each other.

x is transposed/block-permuted on the host during sharding so the
contraction dim lands on the SBUF partition axis and every DMA chunk is
contiguous.
"""

import numpy as np

import concourse.bass as bass
import concourse.mybir as mybir
import concourse.tile as tile
from concourse import bacc
from concourse.bass_utils import run_bass_kernel_spmd

BATCH, IN_DIM, N_RULES = 8192, 4096, 2048
N_CORES = 8
M_LOCAL = BATCH // N_CORES  # 1024 batch rows per core

P = 128            # SBUF partitions / matmul tile edge
N_TILE = 512       # moving free dim per matmul output (= one f32 PSUM bank)
NB_W = 1024        # n-block width
NB = N_RULES // NB_W        # 2 n-blocks
NT = NB_W // N_TILE         # 2 psum tiles per chain
KT = IN_DIM // P            # 32 k-tiles
KP = KT // 2                # 16 k-pairs (DoubleRow consumes 2 per matmul)
MT = M_LOCAL // P           # 8 batch chunks per core

F32 = mybir.dt.float32
FP8 = mybir.dt.float8e4
ALU = mybir.AluOpType
DR = mybir.MatmulPerfMode.DoubleRow


def _body(tc: tile.TileContext, out: bass.AP, xb: bass.AP, w: bass.AP):
    nc = tc.nc
    with (
        tc.tile_pool(name="sb", bufs=1) as sb,
        tc.tile_pool(name="ps", bufs=3, space="PSUM") as ps,
    ):
        # Resident binarized operands.
        # s2[m][kk]: [128, 2, 128] fp8 = 1[x<1] for batch chunk m, k-pair kk
        # wb2[kk]:   [128, 2, NB_W] fp8 = 1[W>.5], double-buffered across nb
        s2 = [[sb.tile([P, 2, P], FP8, tag=f"s{m}_{kk}", bufs=1,
                       name=f"s{m}_{kk}") for kk in range(KP)]
              for m in range(MT)]

        def load_w_pair(nb, kk):
            """DMA + binarize one k-pair of the W block for n-block nb."""
            wb = sb.tile([P, 2, NB_W], FP8, tag=f"wb{kk}", bufs=2,
                         name=f"wb{nb}_{kk}")
            for j in (0, 1):
                k = 2 * kk + j
                wf = sb.tile([P, NB_W], F32, tag="wf", bufs=6, name=f"wf{nb}_{k}")
                nc.sync.dma_start(wf[:], w[k * P:(k + 1) * P,
                                           nb * NB_W:(nb + 1) * NB_W])
                nc.vector.tensor_scalar(wb[:, j, :], wf[:], 0.5, None, ALU.is_gt)
            return wb

        def load_x_pair(m, kk):
            """DMA + binarize one k-pair of batch chunk m."""
            for j in (0, 1):
                k = 2 * kk + j
                xf = sb.tile([P, P], F32, tag="xf", bufs=8, name=f"xf{m}_{k}")
                nc.scalar.dma_start(xf[:], xb[m, k * P:(k + 1) * P, :])
                nc.vector.tensor_scalar(s2[m][kk][:, j, :], xf[:], 1.0,
                                        None, ALU.is_lt)

        def chain(nb, m, wb2):
            """Accumulate + threshold + store one [128, NB_W] output block."""
            accs = [ps.tile([P, N_TILE], F32, tag=f"acc{nt}", bufs=3,
                            name=f"acc{nb}_{m}_{nt}") for nt in range(NT)]
            for kk in range(KP):
                for nt in range(NT):
                    nc.tensor.matmul(
                        accs[nt][:],
                        s2[m][kk][:],
                        wb2[kk][:, :, nt * N_TILE:(nt + 1) * N_TILE],
                        start=(kk == 0),
                        stop=(kk == KP - 1),
                        perf_mode=DR,
                    )
            for nt in range(NT):
                o = sb.tile([P, N_TILE], F32, tag="o", bufs=6,
                            name=f"o{nb}_{m}_{nt}")
                nc.vector.tensor_scalar(o[:], accs[nt][:], 0.0, None, ALU.is_le)
                nc.gpsimd.dma_start(
                    out[m * P:(m + 1) * P,
                        nb * NB_W + nt * N_TILE:nb * NB_W + (nt + 1) * N_TILE],
                    o[:])

        # ---- n-block 0: stream W + first x chunk k-pair-wise (ramp) ----
        wb2_0 = []
        for kk in range(KP):
            wb2_0.append(load_w_pair(0, kk))
            load_x_pair(0, kk)
        chain(0, 0, wb2_0)

        # ---- rest of x, remaining nb0 chains; prefetch W block 1 ----
        wb2_1 = []
        for m in range(1, MT):
            for kk in range(KP):
                load_x_pair(m, kk)
            # spread nb1's W prefetch across the nb0 compute
            while len(wb2_1) < (m * KP) // (MT - 1):
                wb2_1.append(load_w_pair(1, len(wb2_1)))
            chain(0, m, wb2_0)
        while len(wb2_1) < KP:
            wb2_1.append(load_w_pair(1, len(wb2_1)))

        # ---- n-block 1: everything resident ----
        for m in range(MT):
            chain(1, m, wb2_1)


_NC_CACHE = {}


def _get_nc():
    if "nc" not in _NC_CACHE:
        nc = bacc.Bacc("TRN2", target_bir_lowering=False, debug=False,
                       num_devices=N_CORES)
        xb = nc.dram_tensor("xb", [MT, IN_DIM, P], F32, kind="ExternalInput")
        w = nc.dram_tensor("w", [IN_DIM, N_RULES], F32, kind="ExternalInput")
        out = nc.dram_tensor("out", [M_LOCAL, N_RULES], F32,
                             kind="ExternalOutput")
        with tile.TileContext(nc) as tc:
            _body(tc, out.ap(), xb.ap(), w.ap())
        nc.compile()
        _NC_CACHE["nc"] = nc
    return _NC_CACHE["nc"]


def kernel(x: np.ndarray, W: np.ndarray, **run_kwargs) -> np.ndarray:
    assert x.shape == (BATCH, IN_DIM) and W.shape == (IN_DIM, N_RULES)
    x = np.ascontiguousarray(x, dtype=np.float32)
    W = np.ascontiguousarray(W, dtype=np.float32)
    nc = _get_nc()
    in_maps = []
    for c in range(N_CORES):
        xs = x[c * M_LOCAL:(c + 1) * M_LOCAL, :]          # [M_LOCAL, IN_DIM]
        # [MT, IN_DIM, P]: batch chunk -> (contraction, 128 batch cols),
        # so every [128, 128] k-tile DMA chunk is contiguous.
        xbk = np.ascontiguousarray(
            xs.T.reshape(IN_DIM, MT, P).transpose(1, 0, 2))
        in_maps.append({"xb": xbk, "w": W})
    res = run_bass_kernel_spmd(nc, in_maps, core_ids=list(range(N_CORES)),
                               **run_kwargs)
    out = np.concatenate([res.results[c]["out"] for c in range(N_CORES)],
                         axis=0)
    if run_kwargs:
        kernel.last_results = res
    return out


# revision 4
# speedup vs baseline: 1.5576x; 1.4727x over previous
"""Trainium2 kernel for a fuzzy-logic ConjunctionLayer forward pass.

Computes  out = 1[ (1 - x) @ 1[W > 0.5] <= 0 ]  for
x: [8192, 4096] f32, W: [4096, 2048] f32 -> out: [8192, 2048] f32.

Sharding: data-parallel over the batch dim across 8 NeuronCores
(x shard [1024, 4096] per core, W replicated), outputs concatenated.

Math: with x in [0, 1], every term (1-x)*Wb is >= 0, so
  res[m,n] <= 0  <=>  res[m,n] == 0  <=>  no k has (x[m,k] < 1 AND W[k,n] > .5).
The output depends only on the support pattern, so both operands are
binarized on device:
  s  = 1[x < 1],  Wb = 1[W > .5]   (both {0,1}, exact in fp8e4)
  acc = s^T.T @ Wb                 (f32 PSUM accumulation - exact integers)
  out = 1[acc <= 0]
fp8 enables the PE DoubleRow perf mode (2 fp8 weights per cell -> 2x
MACs/cycle, contraction 256 per matmul).

The kernel is DMA-bound (56 MB/core at ~180 GB/s per HWDGE ring), so the
schedule is built around the two HWDGE rings (Sync + Scalar issue queues):
every k-pair step alternates x-slab and W-pair transfers across both
rings, and the 8 batch-chunk accumulation chains (one PSUM bank each)
consume each k-pair the moment it lands. Later n-blocks' W prefetches
ride the same rings; output stores are split between the GPSIMD SWDGE
queue and the rings.

x is transposed on the host during sharding so the contraction dim lands
on the SBUF partition axis for both matmul operands (each x slab DMA is
a contiguous 512 KB block).
"""

import numpy as np

import concourse.bass as bass
import concourse.mybir as mybir
import concourse.tile as tile
from concourse import bacc
from concourse.bass_utils import run_bass_kernel_spmd

BATCH, IN_DIM, N_RULES = 8192, 4096, 2048
N_CORES = 8
M_LOCAL = BATCH // N_CORES  # 1024 batch rows per core

P = 128            # SBUF partitions / matmul tile edge
NB_W = 512         # n-block width (= one f32 PSUM bank)
NB = N_RULES // NB_W        # 4 n-blocks
KT = IN_DIM // P            # 32 k-tiles
KP = KT // 2                # 16 k-pairs (DoubleRow consumes 2 per matmul)
MT = M_LOCAL // P           # 8 batch chunks per core

F32 = mybir.dt.float32
FP8 = mybir.dt.float8e4
ALU = mybir.AluOpType
DR = mybir.MatmulPerfMode.DoubleRow


def _body(tc: tile.TileContext, out: bass.AP, xT: bass.AP, w: bass.AP):
    nc = tc.nc
    rings = (nc.sync, nc.scalar)  # the two HWDGE issue queues
    with (
        tc.tile_pool(name="sb", bufs=1) as sb,
        tc.tile_pool(name="ps", bufs=1, space="PSUM") as ps,
    ):
        # Resident binarized operands.
        s2 = [sb.tile([P, 2, M_LOCAL], FP8, tag=f"s{kk}", bufs=1,
                      name=f"s{kk}") for kk in range(KP)]
        wb2 = [[sb.tile([P, 2, NB_W], FP8, tag=f"wb{nb}_{kk}", bufs=1,
                        name=f"wb{nb}_{kk}") for kk in range(KP)]
               for nb in range(NB)]

        def load_x_pair(kk, ring_flip):
            for j in (0, 1):
                k = 2 * kk + j
                xf = sb.tile([P, M_LOCAL], F32, tag="xf", bufs=4,
                             name=f"xf{k}")
                rings[(j + ring_flip) % 2].dma_start(
                    xf[:], xT[k * P:(k + 1) * P, :])
                nc.vector.tensor_scalar(s2[kk][:, j, :], xf[:], 1.0,
                                        None, ALU.is_lt)

        def load_w_pair(nb, kk, ring_flip):
            n0 = nb * NB_W
            for j in (0, 1):
                k = 2 * kk + j
                wf = sb.tile([P, NB_W], F32, tag="wf", bufs=8,
                             name=f"wf{nb}_{k}")
                rings[(j + ring_flip) % 2].dma_start(
                    wf[:], w[k * P:(k + 1) * P, n0:n0 + NB_W])
                nc.vector.tensor_scalar(wb2[nb][kk][:, j, :], wf[:], 0.5,
                                        None, ALU.is_gt)

        accs = {}

        def mm_step(nb, kk):
            """All 8 batch chains consume k-pair kk of n-block nb."""
            for m in range(MT):
                if kk == 0:
                    accs[m] = ps.tile([P, NB_W], F32, tag=f"acc{m}", bufs=1,
                                      name=f"acc{nb}_{m}")
                nc.tensor.matmul(
                    accs[m][:],
                    s2[kk][:, :, m * P:(m + 1) * P],
                    wb2[nb][kk][:],
                    start=(kk == 0),
                    stop=(kk == KP - 1),
                    perf_mode=DR,
                )

        def epilogue(nb):
            n0 = nb * NB_W
            for m in range(MT):
                o = sb.tile([P, NB_W], F32, tag="o", bufs=8,
                            name=f"o{nb}_{m}")
                nc.vector.tensor_scalar(o[:], accs[m][:], 0.0, None,
                                        ALU.is_le)
                # split stores: even chunks on SWDGE, odd on an HWDGE ring
                eng = nc.gpsimd if m % 2 == 0 else rings[(nb + m) % 2]
                eng.dma_start(out[m * P:(m + 1) * P, n0:n0 + NB_W], o[:])

        # n-block 0: stream x + W k-pair-wise so chains ride the DMA
        for kk in range(KP):
            load_x_pair(kk, ring_flip=0)
            load_w_pair(0, kk, ring_flip=1)
            mm_step(0, kk)
        epilogue(0)

        # n-blocks 1..3: W-only streams, chains consume on arrival
        for nb in range(1, NB):
            for kk in range(KP):
                load_w_pair(nb, kk, ring_flip=kk % 2)
                mm_step(nb, kk)
            epilogue(nb)


_NC_CACHE = {}


def _get_nc():
    if "nc" not in _NC_CACHE:
        nc = bacc.Bacc("TRN2", target_bir_lowering=False, debug=False,
                       num_devices=N_CORES)
        xT = nc.dram_tensor("xT", [IN_DIM, M_LOCAL], F32, kind="ExternalInput")
        w = nc.dram_tensor("w", [IN_DIM, N_RULES], F32, kind="ExternalInput")
        out = nc.dram_tensor("out", [M_LOCAL, N_RULES], F32,
                             kind="ExternalOutput")
        with tile.TileContext(nc) as tc:
            _body(tc, out.ap(), xT.ap(), w.ap())
        nc.compile()
        _NC_CACHE["nc"] = nc
    return _NC_CACHE["nc"]


def kernel(x: np.ndarray, W: np.ndarray, **run_kwargs) -> np.ndarray:
    assert x.shape == (BATCH, IN_DIM) and W.shape == (IN_DIM, N_RULES)
    x = np.ascontiguousarray(x, dtype=np.float32)
    W = np.ascontiguousarray(W, dtype=np.float32)
    nc = _get_nc()
    in_maps = []
    for c in range(N_CORES):
        x_shard_t = np.ascontiguousarray(
            x[c * M_LOCAL:(c + 1) * M_LOCAL, :].T)  # [IN_DIM, M_LOCAL]
        in_maps.append({"xT": x_shard_t, "w": W})
    res = run_bass_kernel_spmd(nc, in_maps, core_ids=list(range(N_CORES)),
                               **run_kwargs)
    out = np.concatenate([res.results[c]["out"] for c in range(N_CORES)],
                         axis=0)
    if run_kwargs:
        kernel.last_results = res
    return out
